# revision 3
# baseline (speedup 1.0000x reference)
"""Trainium2 Bass kernel for nn_CosineLoss (data-parallel over 8 NeuronCores).

loss = -sum_n pred[n, t[n]] / (||pred[n]|| + eps) / N
       + 0.1 * mean_n (1 - ||pred[n]||)^2

Strategy per core (8192 rows x 1000 cols, f32):
  - Stream [128, 8*1000] super-tiles from HBM in 2MB halves.
  - ACT engine: Square per half -> sq scratch; DVE: one segmented
    tensor_reduce per half -> per-row sum of squares (2 ops per half
    instead of per-block activation+bn_stats+fixup chains).
  - GpSimd ap_gather per half + mask-mul + segmented reduce -> the
    per-row target element.
  - Final (two column phases): sqrt -> reciprocal -> g*inv; output
    per-partition partial sums (G, S1=sum norm, S2=sum normsq); the
    host expands (1-norm)^2 = 1 - 2 norm + norm^2 and combines.
    EPS is dropped on device: norms ~ sqrt(1000), so g/(norm+eps)
    differs from g/norm by ~3e-11 relative.
  - Last super-tile streams in 2MB + 1MB + 0.5MB + 0.5MB chunks; its
    last two blocks use Square+accum_out so the tail after the final
    byte is one small activation + a short final chain.
"""

import sys

for _p in ("/root/.axon_site/_ro/trn_rl_repo", "/opt/trn_rl_repo"):
    if _p not in sys.path:
        sys.path.append(_p)

import numpy as np

N = 65536
C = 1000
NCORES = 8
R = N // NCORES          # rows per core
P = 128                  # partitions
NT = R // P              # 64 row-blocks per core
SUP = 8                  # row-blocks per super-tile
NSUP = NT // SUP         # 8 super-tiles per core
EPS = 1e-9
NORM_FACTOR = 0.1

_STATE = {}


def _build_program():
    import concourse.bacc as bacc
    import concourse.bass as bass
    import concourse.mybir as mybir
    import concourse.tile as tile

    f32 = mybir.dt.float32
    i16 = mybir.dt.int16
    AF = mybir.ActivationFunctionType
    ALU = mybir.AluOpType
    AX = mybir.AxisListType

    nc = bacc.Bacc(
        "TRN2",
        target_bir_lowering=False,
        debug=False,
        enable_asserts=False,
        num_devices=NCORES,
    )

    pred_d = nc.dram_tensor("pred", [R, C], f32, kind="ExternalInput").ap()
    tgt_d = nc.dram_tensor("tgt", [P, NT], i16, kind="ExternalInput").ap()
    m128_d = nc.dram_tensor("m128", [P, 4 * 16], f32, kind="ExternalInput").ap()
    out_d = nc.dram_tensor("out", [P, 6], f32, kind="ExternalOutput").ap()

    # [R, C] viewed as [p, supertile, r*c]: row = s*1024 + p*8 + r, i.e.
    # each partition holds 8 consecutive rows per super-tile, so each
    # partition's DMA run is 32KB contiguous.
    pred_v = pred_d.rearrange("(s p r) c -> p s (r c)", p=P, r=SUP)

    with tile.TileContext(nc) as tc:
        from contextlib import ExitStack

        with ExitStack() as ctx:
            data_pool = ctx.enter_context(tc.tile_pool(name="data", bufs=3))
            sq_pool = ctx.enter_context(tc.tile_pool(name="sq", bufs=2))
            g16_pool = ctx.enter_context(tc.tile_pool(name="g16", bufs=2))
            junk_pool = ctx.enter_context(tc.tile_pool(name="junk", bufs=2))
            scr_pool = ctx.enter_context(tc.tile_pool(name="scr", bufs=2))
            persist = ctx.enter_context(tc.tile_pool(name="persist", bufs=1))

            tgt_t = persist.tile([P, NT], i16)
            m128_t = persist.tile([P, 4 * 16], f32)
            dummy = persist.tile([P, 1], f32)
            dummy2 = persist.tile([P, 1], f32)
            sumsq = persist.tile([P, NT], f32)
            gath = persist.tile([P, NT], f32)
            norms = persist.tile([P, NT], f32)
            inv = persist.tile([P, NT], f32)
            g2 = persist.tile([P, NT], f32)
            out_t = persist.tile([P, 6], f32)

            # Preload the sqrt_and_others ACT table set while the first
            # super-tile streams; Square is a filler in every set, so no
            # mid-kernel set switch.
            nc.gpsimd.memset(dummy[:], 1.0)
            nc.scalar.activation(dummy2[:], dummy[:], AF.Sqrt)

            def emit_half(s, h, data):
                """Square + row-sum + gather for 4-block half h of tile s."""
                j0 = SUP * s + 4 * h
                sq = sq_pool.tile([P, SUP * C], f32, tag="sq")
                nc.scalar.activation(
                    sq[:, bass.ts(h, 4 * C)],
                    data[:, bass.ts(h, 4 * C)],
                    AF.Square,
                )
                nc.vector.tensor_reduce(
                    sumsq[:, j0 : j0 + 4],
                    sq[:, bass.ts(h, 4 * C)].rearrange("p (b c) -> p b c", c=C),
                    AX.X,
                    ALU.add,
                )
                emit_gather(s, h, data)

            def emit_gather(s, h, data):
                g16 = g16_pool.tile([P, 4 * 16], f32)
                nc.gpsimd.ap_gather(
                    g16[:],
                    data[:, bass.ts(h, 4 * C)],
                    tgt_t[:, bass.ts(2 * s + h, 4)],
                    channels=P,
                    num_elems=4 * C,
                    d=1,
                    num_idxs=4 * 16,
                )
                gm = junk_pool.tile([P, 4 * 16], f32)
                nc.vector.tensor_mul(gm[:], g16[:], m128_t[:])
                nc.vector.tensor_reduce(
                    gath[:, bass.ts(2 * s + h, 4)],
                    gm[:].rearrange("p (b i) -> p b i", i=16),
                    AX.X,
                    ALU.add,
                )

            def emit_final(c0, c1, phase):
                """Per-partition partials over ss cols [c0, c1):
                out[:, 3p+0] = sum gath*inv, 3p+1 = sum norm, 3p+2 = sum ss."""
                o = 3 * phase
                nc.scalar.activation(norms[:, c0:c1], sumsq[:, c0:c1], AF.Sqrt)
                nc.vector.reciprocal(inv[:, c0:c1], norms[:, c0:c1])
                nc.vector.tensor_mul(g2[:, c0:c1], gath[:, c0:c1], inv[:, c0:c1])
                nc.vector.tensor_reduce(
                    out_t[:, o : o + 1], g2[:, c0:c1], AX.X, ALU.add
                )
                nc.vector.tensor_reduce(
                    out_t[:, o + 1 : o + 2], norms[:, c0:c1], AX.X, ALU.add
                )
                nc.vector.tensor_reduce(
                    out_t[:, o + 2 : o + 3], sumsq[:, c0:c1], AX.X, ALU.add
                )
                nc.sync.dma_start(out_d[:, o : o + 3], out_t[:, o : o + 3])

            for s in range(NSUP):
                data = data_pool.tile([P, SUP * C], f32)
                if s < NSUP - 1:
                    # Two 2MB halves per super-tile.
                    for h in range(2):
                        nc.sync.dma_start(
                            data[:, bass.ts(h, 4 * C)],
                            pred_v[:, s, bass.ts(h, 4 * C)],
                        )
                    if s == 0:
                        # Small loads go behind the first streaming DMA.
                        nc.sync.dma_start(tgt_t[:], tgt_d[:])
                        nc.sync.dma_start(m128_t[:], m128_d[:])
                    for h in range(2):
                        emit_half(s, h, data)
                else:
                    # Last super-tile: 1MB,1MB,1MB,0.5MB,0.5MB chunks and
                    # per-block Square+accum_out, so squaring tracks DMA
                    # arrival and the post-stream tail is one small block.
                    for c_lo, c_hi in ((0, 2), (2, 4), (4, 6), (6, 7), (7, 8)):
                        nc.sync.dma_start(
                            data[:, c_lo * C : c_hi * C],
                            pred_v[:, s, c_lo * C : c_hi * C],
                        )
                    j0 = SUP * s

                    def sq_acc(b):
                        scr = scr_pool.tile([P, C], f32, tag=f"scr{b % 2}")
                        nc.scalar.activation(
                            scr[:],
                            data[:, b * C : (b + 1) * C],
                            AF.Square,
                            accum_out=sumsq[:, j0 + b : j0 + b + 1],
                        )

                    for b in range(4):
                        sq_acc(b)
                    emit_gather(s, 0, data)
                    # Phase 1 over the first 7 super-tiles while the last
                    # chunks stream.
                    emit_final(0, SUP * (NSUP - 1), 0)
                    for b in range(4, SUP):
                        sq_acc(b)
                    emit_gather(s, 1, data)

            # Phase 2: last super-tile's 8 columns.
            emit_final(SUP * (NSUP - 1), NT, 1)

    nc.compile()
    return nc


def _host_shard(prediction, target):
    """Build per-core input maps."""
    prediction = np.asarray(prediction, dtype=np.float32)
    target = np.asarray(target)

    m128 = (
        (np.arange(4 * 16)[None, :] % 16) == (np.arange(P)[:, None] % 16)
    ).astype(np.float32)

    in_maps = []
    for k in range(NCORES):
        pred_k = np.ascontiguousarray(prediction[k * R : (k + 1) * R])
        t_k = target[k * R : (k + 1) * R].astype(np.int64)
        # Device row layout: row = s*1024 + p*8 + r  (s = super-tile,
        # p = partition, r = sub-row). Column j = 8*s + r of tgt/sumsq.
        # Gather offset within a 4-block half is (r % 4) * C.
        tk = t_k.reshape(NSUP, P, SUP)  # [s, p, r]
        tk = np.transpose(tk, (1, 0, 2)).reshape(P, NT)  # [p, 8s+r]
        off = (np.arange(NT) % 4) * C  # [64]
        tgt_k = (tk + off[None, :]).astype(np.int16)
        in_maps.append({"pred": pred_k, "tgt": tgt_k, "m128": m128})
    return in_maps


def _combine(results):
    """results: list of {'out': [128, 6]} per core -> scalar f32 loss.

    out cols: (G, S1, S2) for phase 1 + (G, S1, S2) for phase 2 where
    G = sum gath*inv, S1 = sum norm, S2 = sum norm^2 per partition.
    """
    outs = np.stack([np.asarray(r["out"], dtype=np.float64) for r in results])
    G = outs[:, :, 0].sum() + outs[:, :, 3].sum()
    S1 = outs[:, :, 1].sum() + outs[:, :, 4].sum()
    S2 = outs[:, :, 2].sum() + outs[:, :, 5].sum()
    NL = N - 2.0 * S1 + S2  # sum (1 - norm)^2
    loss = -G / N + NORM_FACTOR * (NL / N)
    return np.float32(loss)


def get_nc():
    if "nc" not in _STATE:
        _STATE["nc"] = _build_program()
    return _STATE["nc"]


def _get_runner():
    """Cached jitted shard_map runner (mirrors bass2jax.run_bass_via_pjrt,
    but reusable across kernel() calls without re-lowering)."""
    if "runner" in _STATE:
        return _STATE["runner"]

    import jax
    from jax.experimental.shard_map import shard_map
    from jax.sharding import Mesh, PartitionSpec

    import concourse.mybir as mybir
    from concourse import bass2jax

    nc = get_nc()
    bass2jax.install_neuronx_cc_hook()

    partition_name = nc.partition_id_tensor.name if nc.partition_id_tensor else None
    in_names, out_names, out_avals, zero_outs = [], [], [], []
    for alloc in nc.m.functions[0].allocations:
        if not isinstance(alloc, mybir.MemoryLocationSet):
            continue
        name = alloc.memorylocations[0].name
        if alloc.kind == "ExternalInput":
            if name != partition_name:
                in_names.append(name)
        elif alloc.kind == "ExternalOutput":
            out_names.append(name)
            shape = tuple(alloc.tensor_shape)
            dtype = mybir.dt.np(alloc.dtype)
            out_avals.append(jax.core.ShapedArray(shape, dtype))
            zero_outs.append(np.zeros(shape, dtype))
    n_params = len(in_names)
    n_outs = len(out_avals)
    all_in = in_names + out_names + ([partition_name] if partition_name else [])

    def _body(*args):
        operands = list(args)
        if partition_name is not None:
            operands.append(bass2jax.partition_id_tensor())
        outs = bass2jax._bass_exec_p.bind(
            *operands,
            out_avals=tuple(out_avals),
            in_names=tuple(all_in),
            out_names=tuple(out_names),
            lowering_input_output_aliases=(),
            sim_require_finite=True,
            sim_require_nnan=True,
            nc=nc,
        )
        return tuple(outs)

    devices = jax.devices()[:NCORES]
    mesh = Mesh(np.asarray(devices), ("core",))
    sharded = jax.jit(
        shard_map(
            _body,
            mesh=mesh,
            in_specs=(PartitionSpec("core"),) * (n_params + n_outs),
            out_specs=(PartitionSpec("core"),) * len(out_names),
            check_rep=False,
        ),
        donate_argnums=tuple(range(n_params, n_params + n_outs)),
        keep_unused=True,
    )

    def run(in_maps):
        concat_in = [
            np.concatenate([np.asarray(in_maps[c][n]) for c in range(NCORES)], axis=0)
            for n in in_names
        ]
        concat_zeros = [
            np.zeros((NCORES * z.shape[0], *z.shape[1:]), z.dtype) for z in zero_outs
        ]
        out_arrs = sharded(*concat_in, *concat_zeros)
        return [
            {
                name: np.asarray(out_arrs[i]).reshape(NCORES, *out_avals[i].shape)[c]
                for i, name in enumerate(out_names)
            }
            for c in range(NCORES)
        ]

    _STATE["runner"] = run
    return run


def kernel(prediction, target):
    in_maps = _host_shard(prediction, target)
    results = _get_runner()(in_maps)
    return _combine(results)


# revision 6
# speedup vs baseline: 1.1789x; 1.1789x over previous
"""Trainium2 Bass kernel for nn_CosineLoss (data-parallel over 8 NeuronCores).

loss = -sum_n pred[n, t[n]] / (||pred[n]|| + eps) / N
       + 0.1 * mean_n (1 - ||pred[n]||)^2

Strategy per core (8192 rows x 1000 cols, f32):
  - Stream [128, 8*1000] super-tiles from HBM in 2MB halves.
  - ACT engine: Square per half -> sq scratch; DVE: one segmented
    tensor_reduce per half -> per-row sum of squares (2 ops per half
    instead of per-block activation+bn_stats+fixup chains).
  - GpSimd ap_gather per half + mask-mul + segmented reduce -> the
    per-row target element.
  - Final (two column phases): sqrt -> reciprocal -> g*inv; output
    per-partition partial sums (G, S1=sum norm, S2=sum normsq); the
    host expands (1-norm)^2 = 1 - 2 norm + norm^2 and combines.
    EPS is dropped on device: norms ~ sqrt(1000), so g/(norm+eps)
    differs from g/norm by ~3e-11 relative.
  - Last super-tile streams in 2MB + 1MB + 0.5MB + 0.5MB chunks; its
    last two blocks use Square+accum_out so the tail after the final
    byte is one small activation + a short final chain.
"""

import sys

for _p in ("/root/.axon_site/_ro/trn_rl_repo", "/opt/trn_rl_repo"):
    if _p not in sys.path:
        sys.path.append(_p)

import numpy as np

N = 65536
C = 1000
NCORES = 8
R = N // NCORES          # rows per core
P = 128                  # partitions
NT = R // P              # 64 row-blocks per core
SUP = 8                  # row-blocks per super-tile
NSUP = NT // SUP         # 8 super-tiles per core
EPS = 1e-9
NORM_FACTOR = 0.1

_STATE = {}


def _build_program():
    import concourse.bacc as bacc
    import concourse.bass as bass
    import concourse.mybir as mybir
    import concourse.tile as tile

    f32 = mybir.dt.float32
    i16 = mybir.dt.int16
    AF = mybir.ActivationFunctionType
    ALU = mybir.AluOpType
    AX = mybir.AxisListType

    nc = bacc.Bacc(
        "TRN2",
        target_bir_lowering=False,
        debug=False,
        enable_asserts=False,
        num_devices=NCORES,
    )

    pred_d = nc.dram_tensor("pred", [R, C], f32, kind="ExternalInput").ap()
    tgt_d = nc.dram_tensor("tgt", [P, NT], i16, kind="ExternalInput").ap()
    m128_d = nc.dram_tensor("m128", [P, 4 * 16], f32, kind="ExternalInput").ap()
    out_d = nc.dram_tensor("out", [P, 6], f32, kind="ExternalOutput").ap()

    # [R, C] viewed as [p, supertile, r*c]: row = s*1024 + p*8 + r, i.e.
    # each partition holds 8 consecutive rows per super-tile, so each
    # partition's DMA run is 32KB contiguous.
    pred_v = pred_d.rearrange("(s p r) c -> p s (r c)", p=P, r=SUP)

    with tile.TileContext(nc) as tc:
        from contextlib import ExitStack

        with ExitStack() as ctx:
            data_pool = ctx.enter_context(tc.tile_pool(name="data", bufs=3))
            sq_pool = ctx.enter_context(tc.tile_pool(name="sq", bufs=2))
            g16_pool = ctx.enter_context(tc.tile_pool(name="g16", bufs=2))
            junk_pool = ctx.enter_context(tc.tile_pool(name="junk", bufs=2))
            scr_pool = ctx.enter_context(tc.tile_pool(name="scr", bufs=2))
            persist = ctx.enter_context(tc.tile_pool(name="persist", bufs=1))

            tgt_t = persist.tile([P, NT], i16)
            m128_t = persist.tile([P, 4 * 16], f32)
            dummy = persist.tile([P, 1], f32)
            dummy2 = persist.tile([P, 1], f32)
            sumsq = persist.tile([P, NT], f32)
            gath = persist.tile([P, NT], f32)
            norms = persist.tile([P, NT], f32)
            inv = persist.tile([P, NT], f32)
            g2 = persist.tile([P, NT], f32)
            out_t = persist.tile([P, 6], f32)

            # Preload the sqrt_and_others ACT table set while the first
            # super-tile streams; Square is a filler in every set, so no
            # mid-kernel set switch.
            nc.gpsimd.memset(dummy[:], 1.0)
            nc.scalar.activation(dummy2[:], dummy[:], AF.Sqrt)

            def emit_sq_reduce(s, data, b_lo, b_hi, sq):
                """Square blocks [b_lo, b_hi) + segmented row-sum reduce."""
                j0 = SUP * s
                nc.scalar.activation(
                    sq[:, b_lo * C : b_hi * C],
                    data[:, b_lo * C : b_hi * C],
                    AF.Square,
                )
                nc.vector.tensor_reduce(
                    sumsq[:, j0 + b_lo : j0 + b_hi],
                    sq[:, b_lo * C : b_hi * C].rearrange(
                        "p (b c) -> p b c", c=C
                    ),
                    AX.X,
                    ALU.add,
                )

            def emit_sq_acc(s, data, b):
                """Per-block Square with accumulator row-sum readout."""
                scr = scr_pool.tile([P, C], f32, tag=f"scr{b % 2}")
                nc.scalar.activation(
                    scr[:],
                    data[:, b * C : (b + 1) * C],
                    AF.Square,
                    accum_out=sumsq[:, SUP * s + b : SUP * s + b + 1],
                )

            def emit_gather(s, h, data):
                g16 = g16_pool.tile([P, 4 * 16], f32)
                nc.gpsimd.ap_gather(
                    g16[:],
                    data[:, bass.ts(h, 4 * C)],
                    tgt_t[:, bass.ts(2 * s + h, 4)],
                    channels=P,
                    num_elems=4 * C,
                    d=1,
                    num_idxs=4 * 16,
                )
                gm = junk_pool.tile([P, 4 * 16], f32)
                nc.vector.tensor_mul(gm[:], g16[:], m128_t[:])
                nc.vector.tensor_reduce(
                    gath[:, bass.ts(2 * s + h, 4)],
                    gm[:].rearrange("p (b i) -> p b i", i=16),
                    AX.X,
                    ALU.add,
                )

            def emit_final(c0, c1, phase):
                """Per-partition partials over ss cols [c0, c1):
                out[:, 3p+0] = sum gath*inv, 3p+1 = sum norm, 3p+2 = sum ss."""
                o = 3 * phase
                nc.scalar.activation(norms[:, c0:c1], sumsq[:, c0:c1], AF.Sqrt)
                nc.vector.reciprocal(inv[:, c0:c1], norms[:, c0:c1])
                nc.vector.tensor_mul(g2[:, c0:c1], gath[:, c0:c1], inv[:, c0:c1])
                nc.vector.tensor_reduce(
                    out_t[:, o : o + 1], g2[:, c0:c1], AX.X, ALU.add
                )
                nc.vector.tensor_reduce(
                    out_t[:, o + 1 : o + 2], norms[:, c0:c1], AX.X, ALU.add
                )
                nc.vector.tensor_reduce(
                    out_t[:, o + 2 : o + 3], sumsq[:, c0:c1], AX.X, ALU.add
                )
                nc.sync.dma_start(out_d[:, o : o + 3], out_t[:, o : o + 3])

            for s in range(NSUP):
                data = data_pool.tile([P, SUP * C], f32)
                last = s == NSUP - 1
                # Blocks 0-3: bulk Square + segmented reduce (half-rate ACT
                # + idle DVE); blocks 4-7: per-block Square+accum so the
                # squaring of late-arriving chunks tracks DMA arrival.
                # The last tile gets finer chunks so its final accum block
                # lands right behind the stream's last byte.
                if last:
                    chunks = ((0, 2), (2, 4), (4, 6), (6, 7), (7, 8))
                else:
                    chunks = ((0, 4), (4, 6), (6, 8))
                for c_lo, c_hi in chunks:
                    nc.sync.dma_start(
                        data[:, c_lo * C : c_hi * C],
                        pred_v[:, s, c_lo * C : c_hi * C],
                    )
                if s == 0:
                    # Small loads go behind the first streaming DMA.
                    nc.sync.dma_start(tgt_t[:], tgt_d[:])
                    nc.sync.dma_start(m128_t[:], m128_d[:])
                sq = sq_pool.tile([P, 4 * C], f32, tag="sq")
                if last:
                    emit_sq_reduce(s, data, 0, 2, sq)
                    emit_sq_reduce(s, data, 2, 4, sq)
                else:
                    emit_sq_reduce(s, data, 0, 4, sq)
                emit_gather(s, 0, data)
                if last:
                    # Phase 1 over the first 7 super-tiles while the last
                    # tile streams (emitted after this tile's dma_starts so
                    # its out-DMA doesn't block the Sync FIFO).
                    emit_final(0, SUP * (NSUP - 1), 0)
                emit_sq_acc(s, data, 4)
                emit_sq_acc(s, data, 5)
                emit_sq_acc(s, data, 6)
                emit_sq_acc(s, data, 7)
                emit_gather(s, 1, data)

            # Phase 2: last super-tile's 8 columns.
            emit_final(SUP * (NSUP - 1), NT, 1)

    nc.compile()
    return nc


def _host_shard(prediction, target):
    """Build per-core input maps."""
    prediction = np.asarray(prediction, dtype=np.float32)
    target = np.asarray(target)

    m128 = (
        (np.arange(4 * 16)[None, :] % 16) == (np.arange(P)[:, None] % 16)
    ).astype(np.float32)

    in_maps = []
    for k in range(NCORES):
        pred_k = np.ascontiguousarray(prediction[k * R : (k + 1) * R])
        t_k = target[k * R : (k + 1) * R].astype(np.int64)
        # Device row layout: row = s*1024 + p*8 + r  (s = super-tile,
        # p = partition, r = sub-row). Column j = 8*s + r of tgt/sumsq.
        # Gather offset within a 4-block half is (r % 4) * C.
        tk = t_k.reshape(NSUP, P, SUP)  # [s, p, r]
        tk = np.transpose(tk, (1, 0, 2)).reshape(P, NT)  # [p, 8s+r]
        off = (np.arange(NT) % 4) * C  # [64]
        tgt_k = (tk + off[None, :]).astype(np.int16)
        in_maps.append({"pred": pred_k, "tgt": tgt_k, "m128": m128})
    return in_maps


def _combine(results):
    """results: list of {'out': [128, 6]} per core -> scalar f32 loss.

    out cols: (G, S1, S2) for phase 1 + (G, S1, S2) for phase 2 where
    G = sum gath*inv, S1 = sum norm, S2 = sum norm^2 per partition.
    """
    outs = np.stack([np.asarray(r["out"], dtype=np.float64) for r in results])
    G = outs[:, :, 0].sum() + outs[:, :, 3].sum()
    S1 = outs[:, :, 1].sum() + outs[:, :, 4].sum()
    S2 = outs[:, :, 2].sum() + outs[:, :, 5].sum()
    NL = N - 2.0 * S1 + S2  # sum (1 - norm)^2
    loss = -G / N + NORM_FACTOR * (NL / N)
    return np.float32(loss)


def get_nc():
    if "nc" not in _STATE:
        _STATE["nc"] = _build_program()
    return _STATE["nc"]


def _get_runner():
    """Cached jitted shard_map runner (mirrors bass2jax.run_bass_via_pjrt,
    but reusable across kernel() calls without re-lowering)."""
    if "runner" in _STATE:
        return _STATE["runner"]

    import jax
    from jax.experimental.shard_map import shard_map
    from jax.sharding import Mesh, PartitionSpec

    import concourse.mybir as mybir
    from concourse import bass2jax

    nc = get_nc()
    bass2jax.install_neuronx_cc_hook()

    partition_name = nc.partition_id_tensor.name if nc.partition_id_tensor else None
    in_names, out_names, out_avals, zero_outs = [], [], [], []
    for alloc in nc.m.functions[0].allocations:
        if not isinstance(alloc, mybir.MemoryLocationSet):
            continue
        name = alloc.memorylocations[0].name
        if alloc.kind == "ExternalInput":
            if name != partition_name:
                in_names.append(name)
        elif alloc.kind == "ExternalOutput":
            out_names.append(name)
            shape = tuple(alloc.tensor_shape)
            dtype = mybir.dt.np(alloc.dtype)
            out_avals.append(jax.core.ShapedArray(shape, dtype))
            zero_outs.append(np.zeros(shape, dtype))
    n_params = len(in_names)
    n_outs = len(out_avals)
    all_in = in_names + out_names + ([partition_name] if partition_name else [])

    def _body(*args):
        operands = list(args)
        if partition_name is not None:
            operands.append(bass2jax.partition_id_tensor())
        outs = bass2jax._bass_exec_p.bind(
            *operands,
            out_avals=tuple(out_avals),
            in_names=tuple(all_in),
            out_names=tuple(out_names),
            lowering_input_output_aliases=(),
            sim_require_finite=True,
            sim_require_nnan=True,
            nc=nc,
        )
        return tuple(outs)

    devices = jax.devices()[:NCORES]
    mesh = Mesh(np.asarray(devices), ("core",))
    sharded = jax.jit(
        shard_map(
            _body,
            mesh=mesh,
            in_specs=(PartitionSpec("core"),) * (n_params + n_outs),
            out_specs=(PartitionSpec("core"),) * len(out_names),
            check_rep=False,
        ),
        donate_argnums=tuple(range(n_params, n_params + n_outs)),
        keep_unused=True,
    )

    def run(in_maps):
        concat_in = [
            np.concatenate([np.asarray(in_maps[c][n]) for c in range(NCORES)], axis=0)
            for n in in_names
        ]
        concat_zeros = [
            np.zeros((NCORES * z.shape[0], *z.shape[1:]), z.dtype) for z in zero_outs
        ]
        out_arrs = sharded(*concat_in, *concat_zeros)
        return [
            {
                name: np.asarray(out_arrs[i]).reshape(NCORES, *out_avals[i].shape)[c]
                for i, name in enumerate(out_names)
            }
            for c in range(NCORES)
        ]

    _STATE["runner"] = run
    return run


def kernel(prediction, target):
    in_maps = _host_shard(prediction, target)
    results = _get_runner()(in_maps)
    return _combine(results)
